# revision 3
# baseline (speedup 1.0000x reference)
"""Trainium2 Bass kernel for the AttentionUnit GNN message-passing block.

Math
----
The nn.Module lifts scalars to `channel` dims with rank-1 weights, so the
whole block collapses to per-batch scalar attention:

    s[b,i,j] = alpha * e[b,i] * v[b,j],     alpha = w_g . w_f
    E = exp(s);  cs[j] = sum_i E[i,j];  rs[i] = sum_j E[i,j]
    out_v = v + beta  * E   @ (v / cs),     beta  = w_h . w_m
    out_e = e + gamma * E^T @ (e / rs),     gamma = w_l . w_n

Since |s| <= m ~ 0.3 (data-dependent, computed at runtime), exp(s) is
replaced by a degree-DEG Chebyshev-interpolated polynomial, which makes E a
rank-(DEG+1) matrix  E = sum_k c_k (e^k)(v^k)^T  that is never materialized:
everything reduces to elementwise powers of the inputs, free-axis
reductions, and per-partition-scalar MACs over [128, 512] tiles.

Layout: pure data parallel over 8 cores, 64 batch rows per core. Each core
stacks its shard as X = [v_rows (partitions 0..63); e_rows (64..127)], so
one op processes both the v- and e- sides. Cross-half "swaps" of [128,1]
scalars and of the final correction use a PE permutation matmul.

The polynomial coefficients depend on the input data, so they are passed as
small input tensors -> the compiled NEFF is input-independent and cached.
"""

import os
from contextlib import ExitStack

import numpy as np

import concourse.bass as bass
import concourse.tile as tile
from concourse import bacc, mybir
from concourse.bass_utils import run_bass_kernel_spmd

B = 512          # batch
D = 512          # dim
N_CORES = 8
BC = B // N_CORES  # 64 batch rows per core
P = 128            # partitions: [v (0..63); e (64..127)]
DEG = int(os.environ.get("ATTN_KERNEL_DEG", "5"))

f32 = mybir.dt.float32
MULT = mybir.AluOpType.mult
ADD = mybir.AluOpType.add


def _build_program(deg: int):
    """Build + compile the single-core Tile program (same NEFF on all 8 cores)."""
    nc = bacc.Bacc(
        "TRN2",
        target_bir_lowering=False,
        debug=False,
        enable_asserts=False,
    )

    xv_d = nc.dram_tensor("xv", [BC, D], f32, kind="ExternalInput")
    xe_d = nc.dram_tensor("xe", [BC, D], f32, kind="ExternalInput")
    # coefs columns: [0:deg] = c_k (k=1..deg); [deg] = c_0 * D;
    #                [deg+1 : 2deg+2] = (gamma|beta) * c_k (k=0..deg)
    coefs_d = nc.dram_tensor("coefs", [P, 2 * deg + 2], f32, kind="ExternalInput")
    perm_d = nc.dram_tensor("perm", [P, P], f32, kind="ExternalInput")
    ov_d = nc.dram_tensor("out_v", [BC, D], f32, kind="ExternalOutput")
    oe_d = nc.dram_tensor("out_e", [BC, D], f32, kind="ExternalOutput")

    with tile.TileContext(nc) as tc, ExitStack() as ctx:
        big = ctx.enter_context(tc.tile_pool(name="big", bufs=1))
        accp = ctx.enter_context(tc.tile_pool(name="accp", bufs=2))
        scr = ctx.enter_context(tc.tile_pool(name="scr", bufs=2))
        small = ctx.enter_context(tc.tile_pool(name="small", bufs=1))
        ps_s = ctx.enter_context(
            tc.tile_pool(name="ps_s", bufs=3, space=bass.MemorySpace.PSUM)
        )
        ps_b = ctx.enter_context(
            tc.tile_pool(name="ps_b", bufs=1, space=bass.MemorySpace.PSUM)
        )

        coefs = small.tile([P, 2 * deg + 2], f32, name="coefs_t")
        nc.sync.dma_start(coefs[:], coefs_d[:])
        perm = small.tile([P, P], f32, name="perm_t")
        nc.sync.dma_start(perm[:], perm_d[:])

        X = big.tile([P, D], f32, name="X")
        nc.sync.dma_start(X[0:BC, :], xv_d[:])
        nc.sync.dma_start(X[BC:P, :], xe_d[:])

        def cden(k):  # c_k, k = 1..deg
            return coefs[:, k - 1 : k]

        def cd0():  # c_0 * D
            return coefs[:, deg : deg + 1]

        def cout(k):  # (gamma|beta) * c_k, k = 0..deg
            return coefs[:, deg + 1 + k : deg + 2 + k]

        # ---- powers P_k = X^k with fused row-sums R_k = sum_j P_k ----
        Pw = {1: X}
        R = {}
        for k in range(1, deg + 1):
            R[k] = small.tile([P, 1], f32, name=f"R{k}")
        nc.vector.tensor_reduce(R[1][:], X[:], axis=mybir.AxisListType.X, op=ADD)
        for k in range(2, deg + 1):
            Pw[k] = big.tile([P, D], f32, name=f"P{k}")
            if k % 2 == 0:
                # even power on the Scalar engine: P_k = Square(P_{k/2})
                nc.scalar.activation(
                    Pw[k][:],
                    Pw[k // 2][:],
                    mybir.ActivationFunctionType.Square,
                    accum_out=R[k][:],
                )
            else:
                # odd power on DVE with fused reduce
                # (scalar_tensor_tensor: out = (in0*1)*in1, accum = sum(out);
                # InstTensorTensorReduce faults on this HW path)
                nc.vector.scalar_tensor_tensor(
                    out=Pw[k][:], in0=Pw[k - 1][:], scalar=1.0, in1=X[:],
                    op0=MULT, op1=MULT, accum_out=R[k][:],
                )

        # ---- b_k = c_k * swap(R_k): PE permutation + ACT scale ----
        bt = {}
        for k in range(1, deg + 1):
            sw = ps_s.tile([P, 1], f32, name=f"swb{k}", tag="swb")
            nc.tensor.matmul(sw[:], perm[:], R[k][:], start=True, stop=True)
            bt[k] = small.tile([P, 1], f32, name=f"b{k}")
            nc.scalar.mul(bt[k][:], sw[:], cden(k))

        # ---- den = sum_k c_k A'_k X^k  (cs on v-half, rs on e-half) ----
        den = accp.tile([P, D], f32, name="den1", tag="den")
        nc.vector.tensor_scalar(
            out=den[:], in0=X[:], scalar1=bt[1][:], scalar2=cd0(), op0=MULT, op1=ADD
        )
        for k in range(2, deg + 1):
            den2 = accp.tile([P, D], f32, name=f"den{k}", tag="den")
            nc.vector.scalar_tensor_tensor(
                out=den2[:], in0=Pw[k][:], scalar=bt[k][:], in1=den[:],
                op0=MULT, op1=ADD,
            )
            den = den2

        # ---- x1 = X / den, with fused Y_0 = sum_j x1 ----
        rcp = accp.tile([P, D], f32, name="rcp", tag="rcp")
        nc.vector.reciprocal_approx_fast(out=rcp[:], in_=den[:])
        YR = {}
        for k in range(0, deg + 1):
            YR[k] = small.tile([P, 1], f32, name=f"YR{k}")
        x1 = big.tile([P, D], f32, name="x1")
        nc.vector.scalar_tensor_tensor(
            out=x1[:], in0=X[:], scalar=1.0, in1=rcp[:],
            op0=MULT, op1=MULT, accum_out=YR[0][:],
        )

        # ---- Y_k = sum_j P_k * x1 ----
        for k in range(1, deg + 1):
            q = scr.tile([P, D], f32, name=f"q{k}", tag="q")
            nc.vector.scalar_tensor_tensor(
                out=q[:], in0=Pw[k][:], scalar=1.0, in1=x1[:],
                op0=MULT, op1=MULT, accum_out=YR[k][:],
            )

        # ---- g_k = (gamma|beta) c_k * swap(Y_k) ----
        gt = {}
        for k in range(0, deg + 1):
            sw = ps_s.tile([P, 1], f32, name=f"swg{k}", tag="swg")
            nc.tensor.matmul(sw[:], perm[:], YR[k][:], start=True, stop=True)
            gt[k] = small.tile([P, 1], f32, name=f"g{k}")
            nc.scalar.mul(gt[k][:], sw[:], cout(k))

        # ---- O = sum_k g_k X^k  ([o_e/..; o_v/..] in swapped halves) ----
        O = accp.tile([P, D], f32, name="O1", tag="O")
        nc.vector.tensor_scalar(
            out=O[:], in0=X[:], scalar1=gt[1][:], scalar2=gt[0][:], op0=MULT, op1=ADD
        )
        for k in range(2, deg + 1):
            O2 = accp.tile([P, D], f32, name=f"O{k}", tag="O")
            nc.vector.scalar_tensor_tensor(
                out=O2[:], in0=Pw[k][:], scalar=gt[k][:], in1=O[:],
                op0=MULT, op1=ADD,
            )
            O = O2

        # ---- OUT = X + swap(O); swap via PE permutation into PSUM ----
        osw = ps_b.tile([P, D], f32, name="osw")
        nc.tensor.matmul(osw[:], perm[:], O[:], start=True, stop=True)
        OUT = big.tile([P, D], f32, name="OUT")
        nc.vector.tensor_tensor(out=OUT[:], in0=X[:], in1=osw[:], op=ADD)

        nc.sync.dma_start(ov_d[:], OUT[0:BC, :])
        nc.sync.dma_start(oe_d[:], OUT[BC:P, :])

    nc.compile()
    return nc


_PROGRAMS: dict[int, object] = {}


def _get_program(deg: int):
    if deg not in _PROGRAMS:
        _PROGRAMS[deg] = _build_program(deg)
    return _PROGRAMS[deg]


def _host_constants(v, e, w_f, w_g, w_h, w_l, w_m, w_n, deg):
    alpha = float(np.dot(w_g.astype(np.float64), w_f.astype(np.float64)))
    beta = float(np.dot(w_h.astype(np.float64), w_m.astype(np.float64)))
    gamma = float(np.dot(w_l.astype(np.float64), w_n.astype(np.float64)))

    # per-batch bound on |s| = |alpha * e_i * v_j|
    m = abs(alpha) * float(
        (np.abs(e).max(axis=1) * np.abs(v).max(axis=1)).max()
    )
    m = max(m * 1.02, 1e-6)

    cheb = np.polynomial.chebyshev.Chebyshev.interpolate(np.exp, deg, domain=[-m, m])
    q = cheb.convert(kind=np.polynomial.polynomial.Polynomial).coef
    q = np.concatenate([q, np.zeros(deg + 1 - len(q))])
    c = np.array([q[k] * alpha**k for k in range(deg + 1)], dtype=np.float64)

    coefs = np.zeros((P, 2 * deg + 2), dtype=np.float32)
    coefs[:, 0:deg] = c[1:]          # c_k, k=1..deg
    coefs[:, deg] = c[0] * D         # c_0 * D
    half = np.where(np.arange(P) < BC, gamma, beta)  # v-half accumulates o_e
    for k in range(deg + 1):
        coefs[:, deg + 1 + k] = half * c[k]

    perm = np.zeros((P, P), dtype=np.float32)
    mm = np.arange(P)
    perm[(mm + BC) % P, mm] = 1.0
    return coefs, perm


def _run(inputs: dict, trace: bool = False):
    v = np.ascontiguousarray(np.asarray(inputs["v_input"], dtype=np.float32))
    e = np.ascontiguousarray(np.asarray(inputs["e_input"], dtype=np.float32))
    assert v.shape == (B, D) and e.shape == (B, D), (v.shape, e.shape)
    ws = {k: np.asarray(inputs[k], dtype=np.float32)
          for k in ("w_f", "w_g", "w_h", "w_l", "w_m", "w_n")}

    coefs, perm = _host_constants(
        v, e, ws["w_f"], ws["w_g"], ws["w_h"], ws["w_l"], ws["w_m"], ws["w_n"], DEG
    )

    nc = _get_program(DEG)
    in_maps = []
    for cidx in range(N_CORES):
        sl = slice(cidx * BC, (cidx + 1) * BC)
        in_maps.append(
            {
                "xv": np.ascontiguousarray(v[sl]),
                "xe": np.ascontiguousarray(e[sl]),
                "coefs": coefs,
                "perm": perm,
            }
        )

    res = run_bass_kernel_spmd(nc, in_maps, list(range(N_CORES)), trace=trace)
    out_v = np.concatenate([res.results[c]["out_v"] for c in range(N_CORES)], axis=0)
    out_e = np.concatenate([res.results[c]["out_e"] for c in range(N_CORES)], axis=0)
    return (out_v, out_e), res


def kernel(**inputs):
    (out_v, out_e), _ = _run(inputs, trace=False)
    return out_v, out_e
